# revision 1
# baseline (speedup 1.0000x reference)
"""Trainium2 Bass kernel for nn_MultiHeadMuonLoRALinear.

Math: out = x @ W^T + bias + sum_h alpha_h * x @ M_h^T, where
M_h = newtonschulz5(B_h @ A_h) and G_h = B_h @ A_h has rank hr=4.

Key algebraic identity: with G = B A (rank hr), every Newton-Schulz
iterate stays in the same row/column space, so X_k = B C_k A for an
hr x hr matrix C_k:
    C_0 = I / (||G||_F + eps),  ||G||_F^2 = tr((B^T B)(A A^T))
    C'  = a C + b (C P C^T) Q C + c (C P C^T Q)^2 C,  P = A A^T, Q = B^T B
Therefore M_h = B_h C_h A_h and the whole LoRA branch collapses to a
rank-16 update:  delta = sum_h alpha_h B_h C_h A_h,  out = x @ (W + delta)^T + bias.

The device kernel computes the single large GEMM (data-parallel over
tokens across 8 cores) with the rank-16 delta folded into W on the
host (0.2% of total FLOPs) and the bias fused into the PSUM->SBUF
copy on the scalar engine. Matmuls run in float32r (fp32 data path,
fp22 multiply) at full PE rate.
"""

import numpy as np

import concourse.bass as bass
import concourse.bacc as bacc
import concourse.mybir as mybir
import concourse.tile as tile
from concourse.bass import ts
from concourse.bass_utils import run_bass_kernel_spmd

N_HEADS = 4
NS_STEPS = 5
NS_EPS = 1e-7
NS_A, NS_B, NS_C = 3.4445, -4.775, 2.0315

N_CORES = 8
P = 128

F32 = mybir.dt.float32
F32R = mybir.dt.float32r


def host_fold_lora(W, bias, lora_A, lora_B):
    """Collapse the per-head Newton-Schulz into hr x hr space (float64)
    and return W_eff = W + sum_h alpha_h B_h C_h A_h (float32)."""
    r, D_in = lora_A.shape
    D_out = lora_B.shape[0]
    hr = r // N_HEADS
    Ah = lora_A.reshape(N_HEADS, hr, D_in).astype(np.float64)
    Bh = lora_B.reshape(D_out, N_HEADS, hr).transpose(1, 0, 2).astype(np.float64)

    AT = np.zeros((r, D_in))   # rows: alpha-weighted C_h A_h per head
    BT = np.zeros((D_out, r))  # cols: B_h per head
    for h in range(N_HEADS):
        A = Ah[h]
        B = Bh[h]
        Pm = A @ A.T
        Qm = B.T @ B
        fro = np.sqrt(np.trace(Qm @ Pm))
        C = np.eye(hr) / (fro + NS_EPS)
        for _ in range(NS_STEPS):
            D = C @ Pm @ C.T
            E = D @ Qm
            C = NS_A * C + NS_B * (E @ C) + NS_C * (E @ (E @ C))
        AT[h * hr:(h + 1) * hr] = fro * (C @ A)
        BT[:, h * hr:(h + 1) * hr] = B
    delta = BT @ AT
    return (W.astype(np.float64) + delta).astype(np.float32)


def build_bass(K, O, T, t_block, phase_a_ot=3, w_dtype=None, warmup=60):
    """Per-core SPMD program: outT[O, T] = (x W_eff^T + bias)^T for this
    core's token shard.

    DRAM layouts (host-prepared, partition-major contiguous):
      x:    [K//128, 128, T]      x_dev[kt, k, t] = x_shard[t, kt*128 + k]
      w:    [O//128, 128, K//128, 128]  w_dev[ot, k, kt, o] = W_eff[ot*128+o, kt*128+k]
      bias: [128, O//128]         bias_dev[o, ot] = bias[ot*128 + o]
      out:  [O, T]                outT

    x streams on the Sync HWDGE ring while W slabs + bias go on the
    Scalar HWDGE ring, so the first weight slab doesn't queue behind
    16MB of x. Phase A k-sweeps the first `phase_a_ot` o-tiles across
    parallel PSUM banks so the PE consumes x tiles as they land; the
    remaining o-tiles run k-contiguous (PE stays warm, one PSUM group
    at a time).
    """
    KT, OT = K // P, O // P
    TB = T // t_block
    A = min(phase_a_ot, OT)
    if w_dtype is None:
        w_dtype = F32R
    nc = bacc.Bacc()

    x_d = nc.declare_dram_parameter("x", [KT, P, T], F32R, isOutput=False)
    w_d = nc.declare_dram_parameter("w", [OT, P, KT, P], w_dtype, isOutput=False)
    b_d = nc.declare_dram_parameter("bias", [P, OT], F32, isOutput=False)
    out_d = nc.declare_dram_parameter("out", [O, T], F32, isOutput=True)

    with tile.TileContext(nc) as tc:
        with (
            tc.tile_pool(name="xpool", bufs=1) as xpool,
            tc.tile_pool(name="cpool", bufs=1) as cpool,
            tc.tile_pool(name="wpool", bufs=A + 1) as wpool,
            tc.tile_pool(name="opool", bufs=3) as opool,
            tc.tile_pool(name="pspool", bufs=8, space="PSUM") as pspool,
        ):
            bias_sb = cpool.tile([P, OT], F32)
            nc.scalar.dma_start(out=bias_sb[:], in_=b_d[:])

            def load_w(ot, engine):
                wt = wpool.tile([P, KT, P], w_dtype, tag="w", name=f"w{ot}")
                engine.dma_start(out=wt[:], in_=w_d[ot])
                return wt

            w_a = [load_w(ot, nc.scalar) for ot in range(A)]

            if warmup:
                # PE warmup: dependency-free matmuls on a memset tile keep
                # the PE busy through the HAM SHORT window while x/W stream
                # in, so phase A runs at 2.4 GHz from its first matmul.
                wu_src = cpool.tile([P, P], F32, name="wu_src")
                nc.vector.memset(wu_src[:], 0.0)
                wu_ps = pspool.tile([P, t_block], F32, tag="ps", name="wu_ps")
                wu_r = wu_src.bitcast(F32R)
                # One accumulation group: no per-matmul semaphores, so
                # the PE issues back-to-back and trips the HAM busy window.
                for i in range(warmup):
                    nc.tensor.matmul(
                        wu_ps[:, :P], lhsT=wu_r[:], rhs=wu_r[:],
                        start=(i == 0), stop=(i == warmup - 1),
                    )

            x_tiles = []
            for kt in range(KT):
                xt = xpool.tile([P, T], F32R, tag=f"x{kt}", name=f"x{kt}")
                nc.sync.dma_start(out=xt[:], in_=x_d[kt])
                x_tiles.append(xt)

            def emit_out(ot, ps_list):
                out_sb = opool.tile([P, T], F32)
                for tb in range(TB):
                    nc.scalar.activation(
                        out_sb[:, ts(tb, t_block)],
                        ps_list[tb][:],
                        mybir.ActivationFunctionType.Identity,
                        bias=bias_sb[:, ot:ot + 1],
                    )
                nc.sync.dma_start(out=out_d[ts(ot, P), :], in_=out_sb[:])

            # Phase A: k-outer sweep over the first A o-tiles in parallel
            # PSUM banks, consuming each x tile as soon as it lands.
            ps_a = [[pspool.tile([P, t_block], F32, tag="ps",
                                 name=f"psA{ot}_{tb}")
                     for tb in range(TB)] for ot in range(A)]
            for kt in range(KT):
                for tb in range(TB):
                    for ot in range(A):
                        nc.tensor.matmul(
                            ps_a[ot][tb][:],
                            lhsT=w_a[ot][:, kt, :],
                            rhs=x_tiles[kt][:, ts(tb, t_block)],
                            start=(kt == 0),
                            stop=(kt == KT - 1),
                        )
            for ot in range(A):
                emit_out(ot, ps_a[ot])

            # Phase B: k-contiguous, one o-tile at a time.
            for ot in range(A, OT):
                wt = load_w(ot, nc.gpsimd)
                ps_list = []
                for tb in range(TB):
                    ps = pspool.tile([P, t_block], F32, tag="ps", name=f"ps{ot}_{tb}")
                    for kt in range(KT):
                        nc.tensor.matmul(
                            ps[:],
                            lhsT=wt[:, kt, :],
                            rhs=x_tiles[kt][:, ts(tb, t_block)],
                            start=(kt == 0),
                            stop=(kt == KT - 1),
                        )
                    ps_list.append(ps)
                emit_out(ot, ps_list)

    nc.compile()
    return nc


def _prep_core_inputs(x2d, W_eff, bias, K, O, T, n_cores, w_np=np.float32):
    """Host-side layout prep: shard tokens, make partition-major layouts."""
    KT, OT = K // P, O // P
    w_dev = np.ascontiguousarray(
        W_eff.reshape(OT, P, KT, P).transpose(0, 3, 2, 1).astype(w_np)
    )  # [ot, k, kt, o]
    bias_dev = np.ascontiguousarray(bias.reshape(OT, P).T)  # [o(part), ot]
    in_maps = []
    for c in range(n_cores):
        xs = x2d[c * T:(c + 1) * T]  # [T, K]
        x_dev = np.ascontiguousarray(xs.reshape(T, KT, P).transpose(1, 2, 0))
        in_maps.append({"x": x_dev, "w": w_dev, "bias": bias_dev})
    return in_maps


W_FP16 = False  # fp16 stationary is rejected by walrus when mixed with f32r


def kernel(x, W, bias, lora_A, lora_B, trace=False, _nc_cache={}):
    x = np.asarray(x, np.float32)
    W = np.asarray(W, np.float32)
    bias = np.asarray(bias, np.float32)
    lora_A = np.asarray(lora_A, np.float32)
    lora_B = np.asarray(lora_B, np.float32)
    B, S, D_in = x.shape
    D_out = bias.shape[0]
    T_total = B * S
    T = T_total // N_CORES

    W_eff = host_fold_lora(W, bias, lora_A, lora_B)
    x2d = np.ascontiguousarray(x.reshape(T_total, D_in))

    w_mb, w_np = (mybir.dt.float16, np.float16) if W_FP16 else (F32R, np.float32)
    key = (D_in, D_out, T, w_mb)
    if key not in _nc_cache:
        _nc_cache[key] = build_bass(D_in, D_out, T, 512, phase_a_ot=3,
                                    w_dtype=w_mb)
    nc = _nc_cache[key]

    in_maps = _prep_core_inputs(x2d, W_eff, bias, D_in, D_out, T, N_CORES,
                                w_np=w_np)
    res = run_bass_kernel_spmd(nc, in_maps, list(range(N_CORES)), trace=trace)

    out = np.empty((T_total, D_out), dtype=np.float32)
    for c in range(N_CORES):
        out[c * T:(c + 1) * T] = res.results[c]["out"].T
    out = out.reshape(B, S, D_out)
    if trace:
        return out, res
    return out



# revision 3
# speedup vs baseline: 1.0857x; 1.0857x over previous
"""Trainium2 Bass kernel for nn_MultiHeadMuonLoRALinear.

Math: out = x @ W^T + bias + sum_h alpha_h * x @ M_h^T, where
M_h = newtonschulz5(B_h @ A_h) and G_h = B_h @ A_h has rank hr=4.

Key algebraic identity: with G = B A (rank hr), every Newton-Schulz
iterate stays in the same row/column space, so X_k = B C_k A for an
hr x hr matrix C_k:
    C_0 = I / (||G||_F + eps),  ||G||_F^2 = tr((B^T B)(A A^T))
    C'  = a C + b (C P C^T) Q C + c (C P C^T Q)^2 C,  P = A A^T, Q = B^T B
Therefore M_h = B_h C_h A_h and the whole LoRA branch collapses to a
rank-16 update:  delta = sum_h alpha_h B_h C_h A_h,  out = x @ (W + delta)^T + bias.

The device kernel computes the single large GEMM (data-parallel over
tokens across 8 cores) with the rank-16 delta folded into W on the
host (0.2% of total FLOPs).

Device strategy (per core, T=1024 tokens, K=O=4096):
  - bf16 x/W/out. PE streams 1 moving column/cycle regardless of dtype
    (fp8 DoubleRow is the only 2x mode but fails the 2e-2 accuracy
    budget), so bf16 keeps the 437us compute floor while halving DMA
    and allowing the full 1024-wide moving operand: one matmul per
    (o-tile, k-tile) amortizes LDWEIGHTS over 1024 cycles.
  - Phase A k-sweeps the first A=4 o-tiles across 4 double-bank PSUM
    groups, consuming x tiles as they stream in on both HWDGE queues
    (evens on sync, odds on scalar) with the phase-A weight slabs
    chunked and interleaved so no MM waits on a whole slab.
  - Phase B runs the remaining o-tiles k-contiguous with W slabs
    prefetched on the gpsimd SWDGE queue.
  - A short PE warmup (12 dep-free matmuls) covers the initial DMA
    window so the HAM clock gate is released by the first real MM.
"""

import numpy as np
import ml_dtypes

import concourse.bass as bass
import concourse.bacc as bacc
import concourse.mybir as mybir
import concourse.tile as tile
from concourse.bass import ts
from concourse.bass_utils import run_bass_kernel_spmd

N_HEADS = 4
NS_STEPS = 5
NS_EPS = 1e-7
NS_A, NS_B, NS_C = 3.4445, -4.775, 2.0315

N_CORES = 8
P = 128

F32 = mybir.dt.float32
BF16 = mybir.dt.bfloat16


def host_fold_lora(W, bias, lora_A, lora_B):
    """Collapse the per-head Newton-Schulz into hr x hr space (float64)
    and return W_eff = W + sum_h alpha_h B_h C_h A_h (float32)."""
    r, D_in = lora_A.shape
    D_out = lora_B.shape[0]
    hr = r // N_HEADS
    Ah = lora_A.reshape(N_HEADS, hr, D_in).astype(np.float64)
    Bh = lora_B.reshape(D_out, N_HEADS, hr).transpose(1, 0, 2).astype(np.float64)

    AT = np.zeros((r, D_in))   # rows: alpha-weighted C_h A_h per head
    BT = np.zeros((D_out, r))  # cols: B_h per head
    for h in range(N_HEADS):
        A = Ah[h]
        B = Bh[h]
        Pm = A @ A.T
        Qm = B.T @ B
        fro = np.sqrt(np.trace(Qm @ Pm))
        C = np.eye(hr) / (fro + NS_EPS)
        for _ in range(NS_STEPS):
            D = C @ Pm @ C.T
            E = D @ Qm
            C = NS_A * C + NS_B * (E @ C) + NS_C * (E @ (E @ C))
        AT[h * hr:(h + 1) * hr] = fro * (C @ A)
        BT[:, h * hr:(h + 1) * hr] = B
    delta = BT @ AT
    return (W.astype(np.float64) + delta).astype(np.float32)


def build_bass(K, O, T, phase_a_ot=4, warmup=12):
    """Per-core SPMD program: outT[O, T] = (x W_eff^T + bias)^T for this
    core's token shard, all-bf16 GEMM with f32 PSUM accumulation.

    DRAM layouts (host-prepared, partition-major contiguous):
      x:    [K//128, 128, T] bf16     x_dev[kt, k, t] = x_shard[t, kt*128 + k]
      w:    [O//128, 128, K//128, 128] bf16  w_dev[ot, k, kt, o] = W_eff[ot*128+o, kt*128+k]
      bias: [128, O//128] f32         bias_dev[o, ot] = bias[ot*128 + o]
      out:  [O, T] bf16               outT
    """
    KT, OT = K // P, O // P
    A = min(phase_a_ot, OT)
    CH = 4              # chunks per phase-A weight slab
    KC = KT // CH       # k-tiles per chunk
    nc = bacc.Bacc()

    x_d = nc.declare_dram_parameter("x", [KT, P, T], BF16, isOutput=False)
    w_d = nc.declare_dram_parameter("w", [OT, P, KT, P], BF16, isOutput=False)
    b_d = nc.declare_dram_parameter("bias", [P, OT], F32, isOutput=False)
    out_d = nc.declare_dram_parameter("out", [O, T], BF16, isOutput=True)

    with tile.TileContext(nc) as tc:
        with (
            tc.tile_pool(name="xpool", bufs=1) as xpool,
            tc.tile_pool(name="cpool", bufs=1) as cpool,
            tc.tile_pool(name="wapool", bufs=1) as wapool,
            tc.tile_pool(name="wbpool", bufs=6) as wbpool,
            tc.tile_pool(name="opool", bufs=4) as opool,
            tc.tile_pool(name="pspool", bufs=4, space="PSUM") as pspool,
        ):
            bias_sb = cpool.tile([P, OT], F32)

            x_tiles = [None] * KT

            def load_x(kt, engine):
                xt = xpool.tile([P, T], BF16, tag=f"x{kt}", name=f"x{kt}")
                engine.dma_start(out=xt[:], in_=x_d[kt])
                x_tiles[kt] = xt

            wa = [[None] * CH for _ in range(A)]

            def load_wa_chunk(a, c, engine):
                wt = wapool.tile([P, KC, P], BF16, tag=f"wa{a}_{c}",
                                 name=f"wa{a}_{c}")
                engine.dma_start(out=wt[:], in_=w_d[a, :, ts(c, KC), :])
                wa[a][c] = wt

            # ---- queue programs (emission order == per-engine queue order)
            # sync HWDGE: first chunks of slabs 0,1 then even x tiles.
            load_wa_chunk(0, 0, nc.sync)
            load_wa_chunk(1, 0, nc.sync)
            for kt in range(0, KT, 2):
                load_x(kt, nc.sync)
            # scalar HWDGE: bias, first chunks of slabs 2,3, then odd x
            # tiles with the later chunks interleaved to match phase-A pace.
            nc.scalar.dma_start(out=bias_sb[:], in_=b_d[:])
            load_wa_chunk(2, 0, nc.scalar)
            load_wa_chunk(3, 0, nc.scalar)
            odd = [kt for kt in range(1, KT, 2)]
            sched = {1: 1, 2: 3, 3: 8}  # chunk c -> insert after this many odd tiles
            done = 0
            for i, kt in enumerate(odd):
                load_x(kt, nc.scalar)
                for c, after in sched.items():
                    if i + 1 == after:
                        for a in range(A):
                            load_wa_chunk(a, c, nc.scalar)
                        done += 1

            if warmup:
                # PE warmup: dependency-free matmuls on a memset tile keep
                # the PE busy through the HAM SHORT window while x/W stream
                # in, so phase A runs at 2.4 GHz from its first matmul.
                wu_src = cpool.tile([P, 512], BF16, name="wu_src")
                nc.vector.memset(wu_src[:], 0.0)
                wu_ps = pspool.tile([P, T], F32, tag="ps", name="wu_ps")
                for i in range(warmup):
                    nc.tensor.matmul(
                        wu_ps[:, :512], lhsT=wu_src[:, :P], rhs=wu_src[:],
                        start=(i == 0), stop=(i == warmup - 1),
                    )

            def emit_out(ot, ps):
                out_sb = opool.tile([P, T], BF16)
                nc.scalar.activation(
                    out_sb[:],
                    ps[:],
                    mybir.ActivationFunctionType.Identity,
                    bias=bias_sb[:, ot:ot + 1],
                )
                eng = nc.sync if ot % 2 == 0 else nc.scalar
                eng.dma_start(out=out_d[ts(ot, P), :], in_=out_sb[:])

            # The ISA caps a matmul's moving AP at 512 elements, so each
            # [128, T=1024] PSUM tile holds two 512-wide accumulation
            # groups; consecutive same-weight matmul pairs let FWL (on for
            # bf16, disabled for fp32r's FP32-HIGH mode) hide LDWEIGHTS.
            TB = T // 512

            # Phase A: k-outer sweep over the first A o-tiles in parallel
            # PSUM groups, consuming each x tile as soon as it lands.
            ps_a = [pspool.tile([P, T], F32, tag="ps", name=f"psA{a}")
                    for a in range(A)]
            for kt in range(KT):
                for a in range(A):
                    for tb in range(TB):
                        nc.tensor.matmul(
                            ps_a[a][:, ts(tb, 512)],
                            lhsT=wa[a][kt // KC][:, kt % KC, :],
                            rhs=x_tiles[kt][:, ts(tb, 512)],
                            start=(kt == 0),
                            stop=(kt == KT - 1),
                        )
            for a in range(A):
                emit_out(a, ps_a[a])

            # Phase B: k-contiguous, one o-tile at a time, W slabs
            # prefetched on the gpsimd SWDGE queue (bufs deep).
            for ot in range(A, OT):
                wt = wbpool.tile([P, KT, P], BF16, tag="wb", name=f"w{ot}")
                nc.gpsimd.dma_start(out=wt[:], in_=w_d[ot])
                ps = pspool.tile([P, T], F32, tag="ps", name=f"ps{ot}")
                for kt in range(KT):
                    for tb in range(TB):
                        nc.tensor.matmul(
                            ps[:, ts(tb, 512)],
                            lhsT=wt[:, kt, :],
                            rhs=x_tiles[kt][:, ts(tb, 512)],
                            start=(kt == 0),
                            stop=(kt == KT - 1),
                        )
                emit_out(ot, ps)

    nc.compile()
    return nc


def _prep_core_inputs(x2d, W_eff, bias, K, O, T, n_cores):
    """Host-side layout prep: shard tokens, make partition-major layouts."""
    KT, OT = K // P, O // P
    w_dev = np.ascontiguousarray(
        W_eff.reshape(OT, P, KT, P).transpose(0, 3, 2, 1)
    ).astype(ml_dtypes.bfloat16)  # [ot, k, kt, o]
    bias_dev = np.ascontiguousarray(bias.reshape(OT, P).T)  # [o(part), ot]
    in_maps = []
    for c in range(n_cores):
        xs = x2d[c * T:(c + 1) * T]  # [T, K]
        x_dev = np.ascontiguousarray(
            xs.reshape(T, KT, P).transpose(1, 2, 0)
        ).astype(ml_dtypes.bfloat16)
        in_maps.append({"x": x_dev, "w": w_dev, "bias": bias_dev})
    return in_maps


def kernel(x, W, bias, lora_A, lora_B, trace=False, _nc_cache={}):
    x = np.asarray(x, np.float32)
    W = np.asarray(W, np.float32)
    bias = np.asarray(bias, np.float32)
    lora_A = np.asarray(lora_A, np.float32)
    lora_B = np.asarray(lora_B, np.float32)
    B, S, D_in = x.shape
    D_out = bias.shape[0]
    T_total = B * S
    T = T_total // N_CORES

    W_eff = host_fold_lora(W, bias, lora_A, lora_B)
    x2d = np.ascontiguousarray(x.reshape(T_total, D_in))

    key = (D_in, D_out, T)
    if key not in _nc_cache:
        _nc_cache[key] = build_bass(D_in, D_out, T)
    nc = _nc_cache[key]

    in_maps = _prep_core_inputs(x2d, W_eff, bias, D_in, D_out, T, N_CORES)
    res = run_bass_kernel_spmd(nc, in_maps, list(range(N_CORES)), trace=trace)

    out = np.empty((T_total, D_out), dtype=np.float32)
    for c in range(N_CORES):
        out[c * T:(c + 1) * T] = res.results[c]["out"].astype(np.float32).T
    out = out.reshape(B, S, D_out)
    if trace:
        return out, res
    return out


# revision 6
# speedup vs baseline: 1.1184x; 1.0301x over previous
"""Trainium2 Bass kernel for nn_MultiHeadMuonLoRALinear.

Math: out = x @ W^T + bias + sum_h alpha_h * x @ M_h^T, where
M_h = newtonschulz5(B_h @ A_h) and G_h = B_h @ A_h has rank hr=4.

Key algebraic identity: with G = B A (rank hr), every Newton-Schulz
iterate stays in the same row/column space, so X_k = B C_k A for an
hr x hr matrix C_k:
    C_0 = I / (||G||_F + eps),  ||G||_F^2 = tr((B^T B)(A A^T))
    C'  = a C + b (C P C^T) Q C + c (C P C^T Q)^2 C,  P = A A^T, Q = B^T B
Therefore M_h = B_h C_h A_h and the whole LoRA branch collapses to a
rank-16 update:  delta = sum_h alpha_h B_h C_h A_h,  out = x @ (W + delta)^T + bias.

The device kernel computes the single large GEMM (data-parallel over
tokens across 8 cores) with the rank-16 delta folded into W on the
host (0.2% of total FLOPs).

Device strategy (per core, T=1024 tokens, K=O=4096):
  - All-bf16 GEMM, f32 PSUM. The PE streams 1 moving column/cycle for
    every dtype >= bf16 (fp8 DoubleRow is the only 2x mode but fails
    the 2e-2 accuracy budget), so the per-core compute floor is
    32*32*2 matmuls x 512 columns = 437us. bf16 halves DMA vs fp32r
    and, critically, enables FWL (fast weight load), which fp32r's
    FP32-HIGH mode disables — LDWEIGHTS becomes fully hidden and the
    steady matmul issue rate drops from 233ns to 216ns.
  - Phase A k-sweeps the first A=4 o-tiles across the four double-bank
    PSUM groups, consuming x tiles in their DMA arrival order. The x
    tiles and phase-A weight chunks are spread across all three DMA
    queues (sync/scalar HWDGE ~100 GB/s each, gpsimd SWDGE ~210 GB/s)
    in a just-in-time order so the PE never starves while x streams.
  - Phase B runs the remaining o-tiles k-contiguous, tb-outer so each
    half's activation+store overlaps the other half's matmuls; W slabs
    are prefetched on the gpsimd queue, throttled by the pool depth.
  - A short PE warmup (dep-free matmuls) covers the fixed ~9us NEFF
    preamble + first-tile DMA window so the HAM clock gate is released
    before the first real matmul.
"""

import numpy as np
import ml_dtypes

import concourse.bass as bass
import concourse.bacc as bacc
import concourse.mybir as mybir
import concourse.tile as tile
from concourse.bass import ts
from concourse.bass_utils import run_bass_kernel_spmd

N_HEADS = 4
NS_STEPS = 5
NS_EPS = 1e-7
NS_A, NS_B, NS_C = 3.4445, -4.775, 2.0315

N_CORES = 8
P = 128

F32 = mybir.dt.float32
BF16 = mybir.dt.bfloat16

A_OT = 4       # phase-A o-tiles (PSUM: 4 groups x 2 banks = all 8 banks)
KC = 4         # k-tiles per phase-A weight chunk
WARMUP = 24


def host_fold_lora(W, bias, lora_A, lora_B):
    """Collapse the per-head Newton-Schulz into hr x hr space (float64)
    and return W_eff = W + sum_h alpha_h B_h C_h A_h (float32)."""
    r, D_in = lora_A.shape
    D_out = lora_B.shape[0]
    hr = r // N_HEADS
    Ah = lora_A.reshape(N_HEADS, hr, D_in).astype(np.float64)
    Bh = lora_B.reshape(D_out, N_HEADS, hr).transpose(1, 0, 2).astype(np.float64)

    AT = np.zeros((r, D_in))   # rows: alpha-weighted C_h A_h per head
    BT = np.zeros((D_out, r))  # cols: B_h per head
    for h in range(N_HEADS):
        A = Ah[h]
        B = Bh[h]
        Pm = A @ A.T
        Qm = B.T @ B
        fro = np.sqrt(np.trace(Qm @ Pm))
        C = np.eye(hr) / (fro + NS_EPS)
        for _ in range(NS_STEPS):
            D = C @ Pm @ C.T
            E = D @ Qm
            C = NS_A * C + NS_B * (E @ C) + NS_C * (E @ (E @ C))
        AT[h * hr:(h + 1) * hr] = fro * (C @ A)
        BT[:, h * hr:(h + 1) * hr] = B
    delta = BT @ AT
    return (W.astype(np.float64) + delta).astype(np.float32)


def build_bass(K, O, T):
    """Per-core SPMD program: outT[O, T] = (x W_eff^T + bias)^T for this
    core's token shard, all-bf16 GEMM with f32 PSUM accumulation.

    DRAM layouts (host-prepared):
      x:    [128, K//128, T] bf16   x_dev[k, kt, t] = x_shard[t, kt*128 + k]
      w:    [O//128, 128, K//128, 128] bf16  w_dev[ot, k, kt, o] = W_eff[ot*128+o, kt*128+k]
      wa:   [CH, 128, A, KC, 128] bf16  phase-A chunk groups:
            wa_dev[c, k, a, kc, o] = W_eff[a*128+o, (c*KC+kc)*128+k]
      bias: [128, O//128] f32       bias_dev[o, ot] = bias[ot*128 + o]
      out:  [O, T] bf16             outT
    """
    KT, OT = K // P, O // P
    A = A_OT
    CH = KT // KC
    TB = T // 512
    nc = bacc.Bacc()

    x_d = nc.declare_dram_parameter("x", [P, KT, T], BF16, isOutput=False)
    w_d = nc.declare_dram_parameter("w", [OT, P, KT, P], BF16, isOutput=False)
    wa_d = nc.declare_dram_parameter("wa", [CH, P, A, KC, P], BF16,
                                     isOutput=False)
    b_d = nc.declare_dram_parameter("bias", [P, OT], F32, isOutput=False)
    out_d = nc.declare_dram_parameter("out", [O, T], BF16, isOutput=True)

    with tile.TileContext(nc) as tc:
        with (
            tc.tile_pool(name="xpool", bufs=1) as xpool,
            tc.tile_pool(name="cpool", bufs=1) as cpool,
            tc.tile_pool(name="wapool", bufs=1) as wapool,
            tc.tile_pool(name="wbpool", bufs=6) as wbpool,
            tc.tile_pool(name="opool", bufs=6) as opool,
            tc.tile_pool(name="pspool", bufs=4, space="PSUM") as pspool,
        ):
            # x access: x_ap(kt) -> [128, T] AP for that k-tile.
            x_ap = [None] * KT

            def load_x(kt, engine):
                xt = xpool.tile([P, T], BF16, tag=f"x{kt}", name=f"x{kt}")
                engine.dma_start(out=xt[:], in_=x_d[:, kt, :])
                x_ap[kt] = xt[:]

            def load_x_batch(k0, k1, engine):
                n = k1 - k0
                xt = xpool.tile([P, n, T], BF16, tag=f"xb{k0}",
                                name=f"xb{k0}")
                engine.dma_start(out=xt[:], in_=x_d[:, k0:k1, :])
                for kt in range(k0, k1):
                    x_ap[kt] = xt[:, kt - k0, :]

            # Phase-A weights: wlhsT(a, kt) -> [128, 128] stationary AP.
            wa_single = {}   # (a, c) -> tile [P, KC, P]
            wa_group = {}    # c -> tile [P, A, KC, P]

            def load_wa(a, c, engine):
                wt = wapool.tile([P, KC, P], BF16, tag=f"wa{a}_{c}",
                                 name=f"wa{a}_{c}")
                engine.dma_start(out=wt[:], in_=wa_d[c, :, a, :, :])
                wa_single[(a, c)] = wt

            def load_wa_group(c, engine):
                wt = wapool.tile([P, A, KC, P], BF16, tag=f"wag{c}",
                                 name=f"wag{c}")
                engine.dma_start(out=wt[:], in_=wa_d[c])
                wa_group[c] = wt

            def wa_lhsT(a, kt):
                c, kc = kt // KC, kt % KC
                if (a, c) in wa_single:
                    return wa_single[(a, c)][:, kc, :]
                return wa_group[c][:, a, kc, :]

            wb = {}          # ot -> slab tile [P, KT, P]

            def load_wb(ot, engine):
                wt = wbpool.tile([P, KT, P], BF16, tag="wb", name=f"w{ot}")
                engine.dma_start(out=wt[:], in_=w_d[ot])
                wb[ot] = wt

            bias_sb = cpool.tile([P, OT], F32)

            # ---- queue programs (emission order == per-engine queue order)
            # Just-in-time supply: phase A consumes (x[kt], wa chunk) pairs
            # at ~1.73us per k-tile from t~14us; each item below lands
            # (at ~100 GB/s HWDGE / ~210 GB/s SWDGE) ahead of its deadline.
            # sync HWDGE:
            load_wa(0, 0, nc.sync)
            load_wa(1, 0, nc.sync)
            load_x(0, nc.sync)
            load_x(2, nc.sync)
            load_wa(0, 1, nc.sync)
            load_wa(1, 1, nc.sync)
            for kt in (4, 6, 8, 10):
                load_x(kt, nc.sync)
            load_x_batch(20, 24, nc.sync)
            load_x_batch(24, 28, nc.sync)
            load_wb(4, nc.sync)
            # scalar HWDGE:
            load_wa(2, 0, nc.scalar)
            load_wa(3, 0, nc.scalar)
            load_x(1, nc.scalar)
            load_x(3, nc.scalar)
            load_wa(2, 1, nc.scalar)
            load_wa(3, 1, nc.scalar)
            for kt in (5, 7, 9, 11):
                load_x(kt, nc.scalar)
            load_x_batch(28, 32, nc.scalar)
            load_wb(5, nc.scalar)
            nc.scalar.dma_start(out=bias_sb[:], in_=b_d[:])
            # gpsimd SWDGE:
            load_wa_group(2, nc.gpsimd)
            load_wa_group(3, nc.gpsimd)
            load_x_batch(12, 16, nc.gpsimd)
            load_wa_group(4, nc.gpsimd)
            load_x_batch(16, 20, nc.gpsimd)
            load_wa_group(5, nc.gpsimd)
            load_wa_group(6, nc.gpsimd)
            load_wa_group(7, nc.gpsimd)

            # PE warmup across the preamble + first-tile DMA window.
            wu_src = cpool.tile([P, 512], BF16, name="wu_src")
            nc.vector.memset(wu_src[:], 0.0)
            wu_ps = pspool.tile([P, T], F32, tag="ps", name="wu_ps")
            for i in range(WARMUP):
                nc.tensor.matmul(
                    wu_ps[:, :512], lhsT=wu_src[:, :P], rhs=wu_src[:],
                    start=(i == 0), stop=(i == WARMUP - 1),
                )

            # Phase A: k-outer sweep over the first A o-tiles in parallel
            # PSUM groups, k-tiles consumed in DMA arrival order.
            K_ORDER = ([0, 1, 2, 3, 4, 5, 6, 7, 12, 8, 13, 9, 14, 10, 15, 11]
                       + list(range(16, KT)))
            assert sorted(K_ORDER) == list(range(KT))
            ps_a = [pspool.tile([P, T], F32, tag="ps", name=f"psA{a}")
                    for a in range(A)]
            for i, kt in enumerate(K_ORDER):
                for a in range(A):
                    for tb in range(TB):
                        nc.tensor.matmul(
                            ps_a[a][:, ts(tb, 512)],
                            lhsT=wa_lhsT(a, kt),
                            rhs=x_ap[kt][:, ts(tb, 512)],
                            start=(i == 0),
                            stop=(i == KT - 1),
                        )

            def emit_half(ot, ps, tb):
                out_sb = opool.tile([P, 512], BF16)
                nc.scalar.activation(
                    out_sb[:],
                    ps[:, ts(tb, 512)],
                    mybir.ActivationFunctionType.Identity,
                    bias=bias_sb[:, ot:ot + 1],
                )
                eng = nc.sync if ot % 2 == 0 else nc.scalar
                eng.dma_start(out=out_d[ts(ot, P), ts(tb, 512)],
                              in_=out_sb[:])

            for a in range(A):
                for tb in range(TB):
                    emit_half(a, ps_a[a], tb)

            # Phase B: k-contiguous, tb-outer so each half's ACT+store
            # overlaps the other half's matmuls; W slabs prefetched on the
            # gpsimd SWDGE queue (pool-depth throttled).
            for ot in range(A, OT):
                if ot not in wb:
                    load_wb(ot, nc.gpsimd)
                wt = wb[ot]
                ps = pspool.tile([P, T], F32, tag="ps", name=f"ps{ot}")
                for tb in range(TB):
                    for kt in range(KT):
                        nc.tensor.matmul(
                            ps[:, ts(tb, 512)],
                            lhsT=wt[:, kt, :],
                            rhs=x_ap[kt][:, ts(tb, 512)],
                            start=(kt == 0),
                            stop=(kt == KT - 1),
                        )
                    emit_half(ot, ps, tb)

    nc.compile()
    return nc


def _prep_core_inputs(x2d, W_eff, bias, K, O, T, n_cores):
    """Host-side layout prep: shard tokens, make partition-major layouts."""
    KT, OT = K // P, O // P
    A, CH = A_OT, KT // KC
    w_bf = W_eff.astype(ml_dtypes.bfloat16)
    w_dev = np.ascontiguousarray(
        w_bf.reshape(OT, P, KT, P).transpose(0, 3, 2, 1)
    )  # [ot, k, kt, o]
    # [c, k, a, kc, o] from W_eff[a*128+o, (c*KC+kc)*128+k]
    wa_dev = np.ascontiguousarray(
        w_bf[:A * P].reshape(A, P, CH, KC, P).transpose(2, 4, 0, 3, 1)
    )
    bias_dev = np.ascontiguousarray(bias.reshape(OT, P).T)  # [o(part), ot]
    in_maps = []
    for c in range(n_cores):
        xs = x2d[c * T:(c + 1) * T]  # [T, K]
        x_dev = np.ascontiguousarray(
            xs.astype(ml_dtypes.bfloat16).reshape(T, KT, P).transpose(2, 1, 0)
        )  # [k, kt, t]
        in_maps.append({"x": x_dev, "w": w_dev, "wa": wa_dev,
                        "bias": bias_dev})
    return in_maps


def kernel(x, W, bias, lora_A, lora_B, trace=False, _nc_cache={}):
    x = np.asarray(x, np.float32)
    W = np.asarray(W, np.float32)
    bias = np.asarray(bias, np.float32)
    lora_A = np.asarray(lora_A, np.float32)
    lora_B = np.asarray(lora_B, np.float32)
    B, S, D_in = x.shape
    D_out = bias.shape[0]
    T_total = B * S
    T = T_total // N_CORES

    W_eff = host_fold_lora(W, bias, lora_A, lora_B)
    x2d = np.ascontiguousarray(x.reshape(T_total, D_in))

    key = (D_in, D_out, T)
    if key not in _nc_cache:
        _nc_cache[key] = build_bass(D_in, D_out, T)
    nc = _nc_cache[key]

    in_maps = _prep_core_inputs(x2d, W_eff, bias, D_in, D_out, T, N_CORES)
    res = run_bass_kernel_spmd(nc, in_maps, list(range(N_CORES)), trace=trace)

    out = np.empty((T_total, D_out), dtype=np.float32)
    for c in range(N_CORES):
        out[c * T:(c + 1) * T] = res.results[c]["out"].astype(np.float32).T
    out = out.reshape(B, S, D_out)
    if trace:
        return out, res
    return out
